# revision 22
# baseline (speedup 1.0000x reference)
"""Trainium2 Bass kernel for single-head attention (no V projection).

Reference computation (per batch b):
    qk   = x @ W_qk.T + b_qk          # [n, 2d]
    q, k = qk[:, :d], qk[:, d:]
    dots[i, j] = k_i . q_j / sqrt(d)
    attn = softmax(dots, axis=-1)
    out[i] = sum_j attn[i, j] * x[j]

Key algebraic folding: dots[i,j] = (Wk x_i + bk).(Wq x_j + bq).  The terms
that depend only on i are constant along the softmax axis j and cancel, so
    softmax_j(dots[i,:]) == softmax_j(x_i^T A x_j + u.x_j),
with A = Wk^T Wq [d,d] and u = Wq^T bk, both input-independent and folded on
the host.  This removes the separate q/k projections: the device computes
    t   = X A + u                     # [n, d]   (stage A; u via ACT bias)
    S^T = X t^T                       # [n, n]   (stage B)
    E   = exp(S^T / 32)               # ACT, no max-subtraction (scores are
                                      #   ~N(0,0.67); exp cannot overflow)
    out = normalize(E^T X)            # (stage C; denominator via a DVE
                                      #   running sum + tiny ones-matmul)
cutting device matmul work from projections + 2 N^2-GEMMs (1552 512-row
matmuls in the unfused form) to 1280.

All matmul operands are bfloat16: measured 216ns per 512-free matmul vs
231ns for float32r, half the SBUF/DMA, end-to-end error ~2.9e-3 scaled vs
the 2e-2 gate (fp8 was measured at 2.7e-2+ in simulation — fails).  Both x
layouts (xt for stages A/B, xv for stage C), t^T, and exp(S^T) stay
SBUF-resident; there is no DRAM spill.  Additional measured-on-HW tuning:
 - ~20 dependency-free warm-up matmuls cover the ~7us DMA head window and
   hold the PE at full p-state (idle drops it to ~630ns/matmul briefly);
 - inputs arrive as few large partition-major DMAs in exact consumption
   order (each descriptor issue costs ~0.7us on the single in-order queue);
 - the softmax denominator (ones-matmul over the f32r running esum +
   reciprocal) is emitted ahead of each output block's j-loop so only the
   two normalize+store halves trail the last matmul.

Sharding: data-parallel over batch b (8 batches -> 8 NeuronCores), no
collectives.
"""
import sys

try:
    import concourse.bass as bass  # noqa: F401
except ImportError:  # pragma: no cover
    sys.path.insert(0, "/opt/trn_rl_repo")

import numpy as np
import ml_dtypes
import concourse.bass as bass
import concourse.mybir as mybir
import concourse.tile as tile
from concourse import bacc
from concourse.bass_utils import run_bass_kernel_spmd
import concourse.bass_utils as _bu

# NOTE: walrus --enable-ldw-opt=true (used by the f32r baseline) is
# incompatible with the explicit InstLdweights that bacc synthesizes for
# multi-wait bf16 matmuls; keep the default (false).

B, N, D = 8, 2048, 1024
NCORES = 8
SCALE = 1.0 / np.sqrt(D)  # 1/32

_NC = None
LAST_RESULTS = None


def _build_nc():
    BF = mybir.dt.bfloat16
    R = mybir.dt.float32r
    F = mybir.dt.float32
    nc = bacc.Bacc("TRN2", target_bir_lowering=False, debug=False, num_devices=NCORES)

    KD = D // 128        # 8 contraction chunks over d
    NJ = N // 128        # 16 key blocks (j)
    CH = 512             # i-chunk width (one PSUM bank)
    NCH = N // CH        # 4 chunks
    NSUB = CH // 128     # 4 row-subblocks per chunk

    # Host-prepared partition-major layouts (lines of 1-2KB for DMA):
    #   xt4[p, k, i] = x[i, k*128+p];  xn4[p, j, d] = x[j*128+p, d];
    #   aw4[p, k, e] = A[k*128+p, e]
    xt4 = nc.dram_tensor("xt4", [128, KD * N], BF, kind="ExternalInput").ap()
    xn4 = nc.dram_tensor("xn4", [128, NJ * D], BF, kind="ExternalInput").ap()
    aw4 = nc.dram_tensor("aw4", [128, KD * D], BF, kind="ExternalInput").ap()
    ub = nc.dram_tensor("ub", [128, KD], F, kind="ExternalInput").ap()
    ones = nc.dram_tensor("ones", [128, 8], R, kind="ExternalInput").ap()
    out = nc.dram_tensor("out", [N, D], F, kind="ExternalOutput").ap()
    xt4r = xt4.rearrange("p (k i) -> p k i", k=KD)
    xn4r = xn4.rearrange("p (j d) -> p j d", j=NJ)
    aw4r = aw4.rearrange("p (k e) -> p k e", k=KD)

    with tile.TileContext(nc) as tc:
        with tc.tile_pool(name="pers", bufs=1) as pers, \
             tc.tile_pool(name="ob", bufs=2) as obp, \
             tc.tile_pool(name="rd", bufs=2) as rdp:

            xt_all = pers.tile([128, KD * N], BF, tag="xt", name="xt_all")
            xtr = xt_all.rearrange("p (k i) -> p k i", k=KD)
            xt = [xtr[:, k] for k in range(KD)]
            xv_all = pers.tile([128, NJ * D], BF, tag="xv", name="xv_all")
            xvr = xv_all.rearrange("p (j d) -> p j d", j=NJ)
            xv = [xvr[:, j] for j in range(NJ)]
            tT = [pers.tile([128, N], BF, tag=f"tT{k}", name=f"tT{k}")
                  for k in range(KD)]
            # f32r so the denominator ones-matmul can consume it directly
            # (the BIR verifier requires fp32r matmul inputs to be written
            # as fp32r; DVE rounds on write).
            esum = pers.tile([128, N], R, tag="esum", name="esum")
            ubt = pers.tile([128, KD], F, tag="ub", name="ubt")
            onesT = pers.tile([128, 8], R, tag="ones", name="onesT")
            warm = pers.tile([128, CH], BF, tag="warm", name="warm")
            nc.vector.memset(warm, 1.0)

            # ---------------- stage A: t^T = A^T X^T (+u) ----------------
            with tc.tile_pool(name="aw", bufs=1) as awp, \
                 tc.tile_pool(name="psA", bufs=2, space="PSUM") as psAp:
                aw_all = awp.tile([128, KD * D], BF, tag="aw", name="aw_all")
                awr = aw_all.rearrange("p (k e) -> p k e", k=KD)

                # PE p-state warm-up: dependency-free matmuls on garbage SBUF
                # fill the otherwise idle DMA head window (~7.5-14.5us: the
                # first A block's aw-m0 + xt-c0 take that long to land) and
                # keep the PE at full clock so the first real matmuls run at
                # 216ns, not at the ~630ns re-ramp rate after an idle gap.
                for w in range(20):
                    wp = psAp.tile([128, CH], F, tag="warm", name="psW")
                    nc.tensor.matmul(wp, warm[:, 0:128], warm,
                                     start=True, stop=True)

                # DMA priority order (one in-order HW queue, ~0.7us per
                # descriptor issue): exactly what the (c0, m) blocks consume,
                # in consumption order; bias early (first ACT drain needs it)
                # but after the first-matmul critical pair; stage-C tensors
                # last.
                nc.sync.dma_start(out=awr[:, :, 0:128], in_=aw4r[:, :, 0:128])
                nc.sync.dma_start(out=xtr[:, :, 0:CH], in_=xt4r[:, :, 0:CH])
                nc.sync.dma_start(out=awr[:, :, 128:256], in_=aw4r[:, :, 128:256])
                nc.sync.dma_start(out=ubt, in_=ub)
                nc.sync.dma_start(out=onesT, in_=ones)
                for m in range(2, KD):
                    nc.sync.dma_start(out=awr[:, :, m * 128:(m + 1) * 128],
                                      in_=aw4r[:, :, m * 128:(m + 1) * 128])
                for c in range(1, NCH):
                    nc.sync.dma_start(out=xtr[:, :, c * CH:(c + 1) * CH],
                                      in_=xt4r[:, :, c * CH:(c + 1) * CH])
                nc.sync.dma_start(out=xvr[:, 0:NJ // 2], in_=xn4r[:, 0:NJ // 2])
                nc.sync.dma_start(out=xvr[:, NJ // 2:NJ], in_=xn4r[:, NJ // 2:NJ])

                for c in range(NCH):
                    cols = slice(c * CH, (c + 1) * CH)
                    for m in range(KD):
                        pt = psAp.tile([128, CH], F, tag="a", name="psA")
                        for k in range(KD):
                            nc.tensor.matmul(
                                pt, awr[:, k, m * 128:(m + 1) * 128],
                                xt[k][:, cols],
                                start=(k == 0), stop=(k == KD - 1))
                        nc.scalar.activation(
                            tT[m][:, cols], pt,
                            mybir.ActivationFunctionType.Identity,
                            bias=ubt[:, m:m + 1], scale=1.0)

            with tc.tile_pool(name="e", bufs=1) as epool:
                ee = [epool.tile([128, N], BF, tag=f"e{j}", name=f"e{j}")
                      for j in range(NJ)]

                # ------------- stage B: S^T strips + exp + esum -------------
                with tc.tile_pool(name="psB", bufs=2, space="PSUM") as psBp:
                    for j in range(NJ):
                        jb = slice(j * 128, (j + 1) * 128)
                        ps = [psBp.tile([128, CH], F, tag=f"b{c}", name=f"psB{c}")
                              for c in range(NCH)]
                        for k in range(KD):
                            for c in range(NCH):
                                nc.tensor.matmul(
                                    ps[c], xt[k][:, jb],
                                    tT[k][:, c * CH:(c + 1) * CH],
                                    start=(k == 0), stop=(k == KD - 1))
                        for c in range(NCH):
                            csl = slice(c * CH, (c + 1) * CH)
                            nc.scalar.activation(
                                ee[j][:, csl], ps[c],
                                mybir.ActivationFunctionType.Exp, scale=SCALE)
                            if j == 0:
                                nc.vector.tensor_copy(esum[:, csl], ee[0][:, csl])
                            else:
                                nc.vector.tensor_add(esum[:, csl], esum[:, csl],
                                                     ee[j][:, csl])

                # ------------- stage C: out rows = attn @ X -------------
                with tc.tile_pool(name="psD", bufs=2, space="PSUM") as psDp, \
                     tc.tile_pool(name="psO", bufs=2, space="PSUM") as psOp:
                    def emit_pd(isl):
                        # denominator: pd only needs esum (ready at stage B
                        # end), so emitted before the j-loop it overlaps it
                        # and the reciprocal is off the drain path.
                        pd = psDp.tile([128, 8], F, tag="pd", name="psD")
                        nc.tensor.matmul(pd, esum[:, isl], onesT,
                                         start=True, stop=True)
                        rden = rdp.tile([128, 1], F, tag="rden", name="rden")
                        nc.vector.reciprocal(rden, pd[:, 0:1])
                        return rden

                    for idx in range(NCH * NSUB):
                        c, sub = divmod(idx, NSUB)
                        i0 = c * CH + sub * 128
                        isl = slice(i0, i0 + 128)
                        # idx 0: pd after the j-loop so C's first PE work is
                        # the j-loop (its PSUM-bank wait is the short one).
                        if idx > 0:
                            rden = emit_pd(isl)
                        p0 = psOp.tile([128, 512], F, tag="p0", name="psO0")
                        p1 = psOp.tile([128, 512], F, tag="p1", name="psO1")
                        for j in range(NJ):
                            lhs = ee[j][:, isl]
                            nc.tensor.matmul(p0, lhs, xv[j][:, 0:512],
                                             start=(j == 0), stop=(j == NJ - 1))
                            nc.tensor.matmul(p1, lhs, xv[j][:, 512:1024],
                                             start=(j == 0), stop=(j == NJ - 1))
                        if idx == 0:
                            rden = emit_pd(isl)
                        # split halves so the last transfer overlaps the
                        # second normalize (input DMAs are long done, so
                        # the sync queue is idle here).
                        ob = obp.tile([128, D], F, tag="ob", name="ob")
                        nc.vector.tensor_scalar_mul(ob[:, 0:512], p0, rden)
                        nc.sync.dma_start(out=out[i0:i0 + 128, 0:512],
                                          in_=ob[:, 0:512])
                        nc.vector.tensor_scalar_mul(ob[:, 512:1024], p1, rden)
                        nc.sync.dma_start(out=out[i0:i0 + 128, 512:1024],
                                          in_=ob[:, 512:1024])

    nc.finalize()
    return nc


def _get_nc():
    global _NC
    if _NC is None:
        _NC = _build_nc()
    return _NC


KD_HOST = D // 128
NJ_HOST = N // 128


def _prep_shared(W_qk, b_qk):
    W_qk = np.ascontiguousarray(W_qk, dtype=np.float32)
    b_qk = np.asarray(b_qk, dtype=np.float32)
    Wq, Wk = W_qk[:D], W_qk[D:]
    bk = b_qk[D:]
    A = (Wk.T @ Wq).astype(ml_dtypes.bfloat16)
    # aw4[p, k, e] = A[k*128+p, e]
    aw4 = np.ascontiguousarray(
        A.reshape(KD_HOST, 128, D).transpose(1, 0, 2).reshape(128, -1))
    u = Wq.T @ bk  # [D]; the bq/i-dependent dot terms cancel in softmax
    ub = np.ascontiguousarray(u.reshape(KD_HOST, 128).T, dtype=np.float32)
    return aw4, ub


def kernel(x: np.ndarray, W_qk: np.ndarray, b_qk: np.ndarray) -> np.ndarray:
    global LAST_RESULTS
    assert x.shape == (B, N, D), x.shape
    nc = _get_nc()

    x = np.ascontiguousarray(x, dtype=np.float32)
    aw4, ub = _prep_shared(W_qk, b_qk)
    ones = np.ones((128, 8), dtype=np.float32)
    in_maps = []
    for c in range(NCORES):
        xb = x[c].astype(ml_dtypes.bfloat16)
        # xt4[p, k, i] = x[i, k*128+p]; xn4[p, j, d] = x[j*128+p, d]
        xt4 = np.ascontiguousarray(
            xb.T.reshape(KD_HOST, 128, N).transpose(1, 0, 2).reshape(128, -1))
        xn4 = np.ascontiguousarray(
            xb.reshape(NJ_HOST, 128, D).transpose(1, 0, 2).reshape(128, -1))
        in_maps.append({
            "xt4": xt4,
            "xn4": xn4,
            "aw4": aw4,
            "ub": ub,
            "ones": ones,
        })

    res = run_bass_kernel_spmd(nc, in_maps, core_ids=list(range(NCORES)))
    LAST_RESULTS = res
    out = np.stack([res.results[c]["out"] for c in range(NCORES)], axis=0)
    return out.astype(np.float32)


if __name__ == "__main__":
    rng = np.random.default_rng(0)
    x = rng.standard_normal((B, N, D), dtype=np.float32)
    limit = float(np.sqrt(6.0 / (D + 2 * D)))
    W = rng.uniform(-limit, limit, size=(2 * D, D)).astype(np.float32)
    b = np.zeros((2 * D,), dtype=np.float32)
    got = kernel(x, W, b)
    print("out", got.shape, got.dtype)
